# revision 16
# baseline (speedup 1.0000x reference)
"""Trainium2 Bass kernel for nn_ControllerNetwork: 2-layer LSTM (hidden=2048)
controller, 8 sequential steps, 10-way heads, categorical sampling.

Strategy: tensor-parallel over the 4*HIDDEN gate dimension across 8 cores
(1024 gate rows per core), all weights SBUF-resident in fp32. Matvecs run
h-stationary on the PE (h chunks as the 1-column stationary operand, W^T
streamed 512 wide as the moving operand, 2-way column tiling). The layer-1
gates of step t and layer-0 gates of step t+1 both depend only on h0(t), so
they ride ONE 8-core AllGather per step (8 collectives total). Every core
then redundantly applies the elementwise cell math for BOTH cells in one
fused op sequence on interleaved [128, 128] tiles, keeping h/c replicated.
A host-side row permutation makes the AllGather output land
partition-fastest, and h_new lands directly in the [128, 32] layout the
next matvecs consume as their stationary columns. The tiny per-step heads
(10x2048 matvec) and the softmax/categorical sampling run on the host.
"""

import sys
import numpy as np

if "/opt/trn_rl_repo" not in sys.path:
    sys.path.insert(0, "/opt/trn_rl_repo")

HIDDEN = 2048
STEPS = 8
CHOICES = 10
N_CORES = 8
GATE = 4 * HIDDEN              # 8192
OSL = GATE // N_CORES          # 1024 gate rows per core
KCH0 = HIDDEN // 128           # 16 contraction chunks, layer 0
KCH1 = 2 * HIDDEN // 128       # 32 contraction chunks, layer 1 ([h0; h1])
NG = 2                         # PE column groups (N=512 each)
NFREE = OSL // NG              # 512 free elems per group
HF = HIDDEN // 128             # 16 free cols of one h/c gate tile

# Layout gate order [i, f, o, g]; original pytorch order is [i, f, g, o].
_GATE_PERM = np.array([0, 1, 3, 2])  # layout block m -> original gate index


def _row_index(core):
    """Full-gate-vector row owned by `core` at matvec free position q.
    q = 512c + 64a + 16m + f (c = PE col group); the hidden sub-index is
    8c + a so one contiguous DMA serves the whole bounce buffer."""
    q = np.arange(OSL)
    c = q // 512
    a = (q % 512) // 64
    m = (q % 64) // 16    # layout gate block
    f = q % 16            # free col of the [128, 16] gate tile
    return _GATE_PERM[m] * HIDDEN + 16 * core + 8 * c + a + 128 * f


def _vec2tile(v):
    """[8192] gate-ordered vector -> [128, 64] tile in the layout the
    AllGather produces: tile[p, 16m + f] = v[perm(m)*2048 + p + 128f]."""
    vr = v.reshape(4, 16, 128)            # [g, f, p]
    return np.ascontiguousarray(
        vr[_GATE_PERM].transpose(2, 0, 1).reshape(128, 64))


def _wslice(Wcat, rows, kch):
    """Rows of Wcat [GATE, C] -> SBUF layout [128, kch*OSL]:
    out[p, OSL*k + q] = Wcat[rows[q], 128k + p]."""
    Ws = Wcat[rows, :]                                     # [OSL, C]
    return np.ascontiguousarray(
        Ws.reshape(OSL, kch, 128).transpose(2, 1, 0).reshape(128, kch * OSL)
    )


def _interleave_bias(b1_tile, b0_tile):
    """Two [128, 64] gate tiles -> one [128, 128] combined-cell tile with
    16-col blocks interleaved [i1 i0 f1 f0 o1 o0 g1 g0]."""
    out = np.empty((128, 128), np.float32)
    for m in range(4):
        out[:, 32 * m : 32 * m + 16] = b1_tile[:, 16 * m : 16 * m + 16]
        out[:, 32 * m + 16 : 32 * m + 32] = b0_tile[:, 16 * m : 16 * m + 16]
    return out


def _build_module(timing=False, repeat=1):
    from concourse import bacc, tile, mybir

    f32 = mybir.dt.float32
    nc = bacc.Bacc("TRN2", target_bir_lowering=False, debug=False,
                   num_devices=N_CORES)

    kind = "Internal" if timing else "ExternalInput"
    w0_in = nc.dram_tensor("w0", [128, KCH0 * OSL], f32, kind=kind).ap()
    w1_in = nc.dram_tensor("w1", [128, KCH1 * OSL], f32, kind=kind).ap()
    # combined bias blocks: col block 0 = init (l1 zeros | l0 step-0),
    # block 1+t = [l1 bias | l0 bias step t+1] (t+1 == 8 -> zeros)
    bc_in = nc.dram_tensor("bc", [128, (STEPS + 1) * 128], f32, kind=kind).ap()
    if timing:
        nc.dram_tensor("x", [1, 1], f32, kind="ExternalInput").ap()
    out = nc.dram_tensor("hstates", [128, STEPS * HF], f32,
                         kind="ExternalOutput").ap()

    with tile.TileContext(nc) as tc:
        with tc.tile_pool(name="wpool", bufs=1) as wpool, \
             tc.tile_pool(name="state", bufs=1) as state, \
             tc.tile_pool(name="work", bufs=2) as work, \
             tc.tile_pool(name="spool", bufs=1) as spool, \
             tc.tile_pool(name="psum", bufs=1, space="PSUM") as psum, \
             tc.tile_pool(name="dram", bufs=2, space="DRAM") as dram:

            w0 = wpool.tile([128, KCH0 * OSL], f32, name="w0t")
            nc.sync.dma_start(w0[:], w0_in[:])
            w1 = wpool.tile([128, KCH1 * OSL], f32, name="w1t")
            nc.sync.dma_start(w1[:], w1_in[:])
            bc = wpool.tile([128, (STEPS + 1) * 128], f32, name="bct")
            nc.sync.dma_start(bc[:], bc_in[:])

            # combined state tiles: cols [0:16] = layer1, [16:32] = layer0
            hc = state.tile([128, 2 * HF], f32, name="hc")
            cc = state.tile([128, 2 * HF], f32, name="cc")
            hstates = state.tile([128, STEPS * HF], f32, name="hstates_sb")
            nc.vector.memset(hc[:], 0.0)
            nc.vector.memset(cc[:], 0.0)

            l0ps = [psum.tile([128, NFREE], f32, name=f"l0ps{c}")
                    for c in range(NG)]
            l1ps = [psum.tile([128, NFREE], f32, name=f"l1ps{c}")
                    for c in range(NG)]

            def matvec(ps, wtile, kch, lhs_col):
                for k in range(kch):
                    lhs = hc[:, lhs_col(k) : lhs_col(k) + 1]
                    for c in range(NG):
                        nc.tensor.matmul(
                            ps[c][32 * c : 32 * c + 1, 0:NFREE],
                            lhs,
                            wtile[:, OSL * k + NFREE * c : OSL * k + NFREE * (c + 1)],
                            start=(k == 0),
                            stop=(k == kch - 1),
                            tile_position=(0, 32 * c),
                        )

            def stage(ps, st, col0):
                # st[32c, col0:col0+512] <- matvec col-group c
                for c in range(NG):
                    nc.vector.tensor_copy(
                        st[32 * c : 32 * c + 1, col0 : col0 + NFREE],
                        ps[c][32 * c : 32 * c + 1, 0:NFREE])

            def cell(gin, bias, tag):
                """Fused dual-layer LSTM cell on interleaved [128,128] gates.
                blocks: [i1 i0 f1 f0 o1 o0 g1 g0] -> updates hc, cc."""
                sg = work.tile([128, 128], f32, name=f"sg_{tag}", tag="sg")
                tmp = work.tile([128, 96], f32, name=f"tmp_{tag}", tag="tmp")
                if bias is not None:
                    gb = work.tile([128, 128], f32, name=f"gb_{tag}", tag="gb")
                    nc.vector.tensor_add(gb[:], gin[:], bias)
                    gin = gb
                nc.scalar.activation(sg[:, 0:96], gin[:, 0:96],
                                     mybir.ActivationFunctionType.Sigmoid)
                nc.scalar.activation(sg[:, 96:128], gin[:, 96:128],
                                     mybir.ActivationFunctionType.Tanh)
                nc.vector.tensor_mul(tmp[:, 0:32], sg[:, 32:64], cc[:])
                nc.vector.tensor_mul(tmp[:, 32:64], sg[:, 0:32], sg[:, 96:128])
                nc.vector.tensor_add(cc[:], tmp[:, 0:32], tmp[:, 32:64])
                nc.scalar.activation(tmp[:, 64:96], cc[:],
                                     mybir.ActivationFunctionType.Tanh)
                nc.vector.tensor_mul(hc[:], sg[:, 64:96], tmp[:, 64:96])

            # init: gates = [zeros | step-0 layer-0 bias] -> h1=c1=0,
            # h0(0), c0(0); zero gates reproduce the zero initial state.
            cell(bc[:, 0:128], None, "init")

            for it in range(STEPS * repeat):
                t = it % STEPS
                # layer-1 step t: contraction [h0(t); h1(t-1)]
                st = spool.tile([128, 2 * NFREE], f32, name=f"st_{it}",
                                tag="st")
                matvec(l1ps, w1, KCH1,
                       lambda k: (HF + k) if k < KCH0 else (k - KCH0))
                stage(l1ps, st, 0)
                # layer-0 step t+1 (h0(8) computed but unused at t=7)
                matvec(l0ps, w0, KCH0, lambda k: HF + k)
                stage(l0ps, st, NFREE)

                # per-rank bounce layout: [c, s, a, m, f] -> ONE DMA each way
                agi = dram.tile([2, 2, 8, 4, 16], f32, name=f"agi_{it}",
                                tag="agi")
                ago = dram.tile([N_CORES, 2, 2, 8, 4, 16], f32,
                                name=f"ago_{it}", tag="ago")
                nc.sync.dma_start(agi[:], st[0 : 32 * NG : 32, :])
                nc.gpsimd.collective_compute(
                    "AllGather", mybir.AluOpType.bypass,
                    replica_groups=[list(range(N_CORES))],
                    ins=[agi[:].opt()], outs=[ago[:].opt()])

                g = work.tile([128, 128], f32, name=f"g_{it}", tag="g")
                gv = g[:].rearrange("p (m x) -> p m x", m=4)
                for s in range(2):
                    # src [8r, 2c, 8a, 4m, 16f] matches dst
                    # [(r c a)=128p, 4m, 16f] element order
                    nc.sync.dma_start(gv[:, :, 16 * s : 16 * s + 16],
                                      ago[:, :, s])

                cell(g, bc[:, 128 * (t + 1) : 128 * (t + 2)], f"c_{it}")
                nc.vector.tensor_copy(hstates[:, HF * t : HF * (t + 1)],
                                      hc[:, 0:HF])

            nc.sync.dma_start(out[:], hstates[:])

    nc.compile()
    return nc


_MODULE_CACHE = {}


def _get_module():
    if "nc" not in _MODULE_CACHE:
        _MODULE_CACHE["nc"] = _build_module()
    return _MODULE_CACHE["nc"]


def host_prep(which_to_use, w_ih0, w_hh0, b_ih0, b_hh0,
              w_ih1, w_hh1, b_ih1, b_hh1):
    """Build the per-core device input maps."""
    xs = np.concatenate([[0.0], which_to_use[:-1].astype(np.float32)])
    bias0 = b_ih0 + b_hh0
    b1_tile = _vec2tile(b_ih1 + b_hh1)
    zero64 = np.zeros((128, 64), np.float32)
    bc_host = np.empty((128, (STEPS + 1) * 128), np.float32)
    bc_host[:, 0:128] = _interleave_bias(zero64,
                                         _vec2tile(bias0 + xs[0] * w_ih0[:, 0]))
    for t in range(STEPS):
        b0n = (_vec2tile(bias0 + xs[t + 1] * w_ih0[:, 0])
               if t + 1 < STEPS else zero64)
        bc_host[:, 128 * (t + 1) : 128 * (t + 2)] = _interleave_bias(
            b1_tile, b0n)
    w1cat = np.concatenate([w_ih1, w_hh1], axis=1)   # [8192, 4096]
    in_maps = []
    for core in range(N_CORES):
        rows = _row_index(core)
        in_maps.append({
            "w0": _wslice(w_hh0, rows, KCH0),
            "w1": _wslice(w1cat, rows, KCH1),
            "bc": bc_host,
        })
    return in_maps


def _host_fallback(which_to_use, w_ih0, w_hh0, b_ih0, b_hh0,
                   w_ih1, w_hh1, b_ih1, b_hh1, head_w, head_b):
    """Pure-host replica of the reference for inputs where which_to_use
    contains -1 (sampled index feeds the recurrence; cannot be scheduled
    ahead of time). Never hit for spec-conformant inputs."""
    import jax
    import jax.numpy as jnp

    cpu = jax.devices("cpu")[0]
    with jax.default_device(cpu):
        keys = jax.random.split(jax.random.key(42), STEPS)
        h0 = c0 = h1 = c1 = jnp.zeros((HIDDEN,), jnp.float32)
        x = jnp.float32(0.0)
        idxs, probas, all_probs = [], [], []

        def lstm(xv, h, c, wi, wh, bi, bh):
            gates = wi @ xv + wh @ h + bi + bh
            i, f, g, o = jnp.split(gates, 4)
            c = jax.nn.sigmoid(f) * c + jax.nn.sigmoid(i) * jnp.tanh(g)
            h = jax.nn.sigmoid(o) * jnp.tanh(c)
            return h, c

        for t in range(STEPS):
            h0, c0 = lstm(x[None], h0, c0, w_ih0, w_hh0, b_ih0, b_hh0)
            h1, c1 = lstm(h0, h1, c1, w_ih1, w_hh1, b_ih1, b_hh1)
            logits = head_w[t] @ h1 + head_b[t]
            probs = jax.nn.softmax(logits)
            samp = jax.random.categorical(keys[t], logits).astype(jnp.int32)
            idx = jnp.where(which_to_use[t] != -1, which_to_use[t], samp)
            idxs.append(idx)
            probas.append(probs[samp])
            all_probs.append(probs)
            x = idx.astype(jnp.float32)
        return (np.array(idxs, np.int32), np.array(probas, np.float32),
                np.array(all_probs, np.float32))


def kernel(which_to_use, w_ih0, w_hh0, b_ih0, b_hh0,
           w_ih1, w_hh1, b_ih1, b_hh1, head_w, head_b):
    from concourse import bass_utils
    import jax
    import jax.numpy as jnp

    wtu = np.asarray(which_to_use, np.int32)
    w_ih0 = np.asarray(w_ih0, np.float32)
    w_hh0 = np.asarray(w_hh0, np.float32)
    b_ih0 = np.asarray(b_ih0, np.float32)
    b_hh0 = np.asarray(b_hh0, np.float32)
    w_ih1 = np.asarray(w_ih1, np.float32)
    w_hh1 = np.asarray(w_hh1, np.float32)
    b_ih1 = np.asarray(b_ih1, np.float32)
    b_hh1 = np.asarray(b_hh1, np.float32)
    head_w = np.asarray(head_w, np.float32)
    head_b = np.asarray(head_b, np.float32)

    if np.any(wtu == -1):
        return _host_fallback(wtu, w_ih0, w_hh0, b_ih0, b_hh0,
                              w_ih1, w_hh1, b_ih1, b_hh1, head_w, head_b)

    in_maps = host_prep(wtu, w_ih0, w_hh0, b_ih0, b_hh0,
                        w_ih1, w_hh1, b_ih1, b_hh1)
    nc = _get_module()
    res = bass_utils.run_bass_kernel_spmd(
        nc, in_maps, core_ids=list(range(N_CORES)))
    hs = res.results[0]["hstates"]                    # [128, 8*16]
    # h1(t)[p + 128f] = hs[p, 16t + f]
    h1s = np.stack([hs[:, HF * t : HF * (t + 1)].T.reshape(HIDDEN)
                    for t in range(STEPS)])           # [8, 2048]
    raw_logits = np.einsum("tch,th->tc", head_w, h1s)

    # Host post-process: identical sampling path to the reference.
    cpu = jax.devices("cpu")[0]
    with jax.default_device(cpu):
        logits = jnp.asarray(raw_logits) + jnp.asarray(head_b)
        probs = np.asarray(jax.nn.softmax(logits, axis=-1))
        keys = jax.random.split(jax.random.key(42), STEPS)
        samp = np.array(
            [int(jax.random.categorical(keys[t], logits[t]))
             for t in range(STEPS)], np.int32)
    idxs = np.where(wtu != -1, wtu, samp).astype(np.int32)
    probas = probs[np.arange(STEPS), samp].astype(np.float32)
    return idxs, probas, probs.astype(np.float32)
